# revision 43
# baseline (speedup 1.0000x reference)
"""Multi-head attention (B=4, S=2048, H=1024, 16 heads) on 8 Trainium2 cores.

Sharding: core c = 2*b + g handles batch b with head-group g (8 heads = 512 of
1024 H-columns).  Each core computes Q/K/V projections for its column slice,
attention for its 8 heads, and a partial output projection against its 512
rows of wo.  The host sums the two partials per batch and adds bo.

Kernel internals (per core):
  - x [2048,1024] f32 is DMA'd in and PE-transposed to xT (bf16 on evac).
  - qT/kT [c,t] computed directly (lhsT = wq chunk, rhs = xT) so attention
    scores come out transposed (scoresT [k,q]) with no further transposes.
  - v computed in natural [t,c] layout (lhsT = xT chunk, rhs = wv), stored
    ones-augmented per head ([.., 65]) so the AV matmul also produces the
    softmax denominator in psum partition 64.
  - exp on ACT (scale folded in); no max-subtraction needed (logits ~N(0,1)).
  - normalization per (head, q-window): raw ctx parked in SBUF bf16, DVE
    reciprocal of the denominator row, partition-broadcast via stride-0 DMA
    from a DRAM scratch, one in-place multiply -- all off the PE critical
    path.
  - O-projection: lhsT = ctxT chunks, rhs = wo chunks, fp32 partial out.
All matmuls run in bf16 with fp32 psum accumulation.  Projection work is
dripped through the attention kc loops as PE fill so the tensor engine
stays saturated (idle gaps reset its clock ramp).
"""
import sys

if "/opt/trn_rl_repo" not in sys.path:
    sys.path.insert(0, "/opt/trn_rl_repo")

import numpy as np

import concourse.bass as bass
import concourse.tile as tile
from concourse import bacc, mybir
from concourse.bass_utils import run_bass_kernel_spmd
from concourse.masks import make_identity

B, S, H = 4, 2048, 1024
NH, HD = 16, 64
G = H // 2            # local H columns per core
NHL = NH // 2         # local heads per core
P = 128
F32 = mybir.dt.float32
BF16 = mybir.dt.bfloat16
SCALE = 1.0 / float(np.sqrt(HD))

TT = S // P           # 16 token tiles
HC = H // P           # 8 contraction chunks for projections
CT = G // P           # 4 c-tiles
KC = S // P           # 16 k chunks
QW = 1024             # q window width in attention
NQH = S // QW         # 2 q windows
NW = NHL * NQH        # 16 (head, q-window) pairs
MM_N = 512            # matmul moving free dim (one psum bank)

_NC_CACHE = None


def _emit(nc, tc, aps):
    x, wq, wk, wv, wo, bq, bk, bv, out, rcp_dram = aps

    import contextlib
    ctx = contextlib.ExitStack()
    with ctx:
        persist = ctx.enter_context(tc.tile_pool(name="persist", bufs=1))

        # ---- persistent sbuf tensors ----
        qT = persist.tile([P, CT, S], BF16)
        kT = persist.tile([P, CT, S], BF16)
        v_aug = persist.tile([P, KC, NHL, HD + 1], BF16)
        ctxT = persist.tile([P, CT, S], BF16)
        wo_sb = persist.tile([P, CT, H], BF16)
        bq_sb = persist.tile([P, CT], F32)
        bk_sb = persist.tile([P, CT], F32)
        bv_row = persist.tile([1, G], BF16)
        ones_col = persist.tile([1, P], BF16)
        identity = persist.tile([P, P], F32)

        make_identity(nc, identity)
        nc.vector.memset(ones_col, 1.0)
        nc.vector.memset(v_aug[:, :, :, HD:HD + 1], 1.0)

        ph1 = ctx.enter_context(tc.tile_pool(name="ph1", bufs=1))
        xT = ph1.tile([P, HC, S], BF16)
        wq_sb = ph1.tile([P, HC, G], BF16)
        wk_sb = ph1.tile([P, HC, G], BF16)
        wv_sb = ph1.tile([P, HC, G], BF16)

        # psum pools for the whole kernel: proj/transpose accumulators share
        # slots (tag "acc"), scores double-buffered, ctx single
        pps = ctx.enter_context(tc.tile_pool(name="proj_ps", bufs=2, space="PSUM"))
        sc_pool = ctx.enter_context(tc.tile_pool(name="sc_ps", bufs=2, space="PSUM"))
        ctx_pool = ctx.enter_context(tc.tile_pool(name="ctx_ps", bufs=1, space="PSUM"))

        # ---- phase 0: loads, x transpose on the PE (evac casts to bf16) ----
        with tc.tile_pool(name="xload", bufs=3) as xload, \
             tc.tile_pool(name="wload", bufs=2) as wload:
            nc.sync.dma_start(out=bq_sb, in_=bq.rearrange("(ct p) -> p ct", p=P))
            nc.sync.dma_start(out=bk_sb, in_=bk.rearrange("(ct p) -> p ct", p=P))
            bv_f = wload.tile([1, G], F32, tag="bias")
            nc.sync.dma_start(out=bv_f, in_=bv.rearrange("(a c) -> a c", a=1))
            nc.vector.tensor_copy(out=bv_row, in_=bv_f)

            for tt in range(TT):
                xt = xload.tile([P, H], F32, tag="x")
                nc.sync.dma_start(out=xt, in_=x[tt * P:(tt + 1) * P, :])
                for hg in range(2):
                    tp = pps.tile([P, 4, P], F32, tag="acc")
                    for j in range(4):
                        nc.tensor.transpose(
                            tp[:, j, :],
                            xt[:, (4 * hg + j) * P:(4 * hg + j + 1) * P],
                            identity)
                    nc.vector.tensor_copy(
                        out=xT[:, 4 * hg:4 * hg + 4, tt * P:(tt + 1) * P],
                        in_=tp)

            # weights: casting DMAs (software DGE converts f32 -> bf16)
            for wap, dst in ((wq, wq_sb), (wk, wk_sb), (wv, wv_sb)):
                nc.gpsimd.dma_start(
                    out=dst, in_=wap.rearrange("(hc p) c -> p hc c", p=P))
            nc.gpsimd.dma_start(
                out=wo_sb, in_=wo.rearrange("(cc p) o -> p cc o", p=P))

        # ---- phases 1+2 interleaved ----
        # PE fill-work is woven into the attention kc loops so the tensor
        # engine never idles (idle gaps reset its clock ramp): the v
        # projection rides inside window 0 (one token-tile per kc), and the
        # qk projections for c-tile ct+1 are dripped as 8-matmul groups
        # through the windows of c-tile ct.
        def emit_v_slice(tt):
            acc = pps.tile([P, G], F32, tag="acc")
            for hc in range(HC):
                nc.tensor.matmul(
                    acc,
                    lhsT=xT[:, hc, tt * P:(tt + 1) * P],
                    rhs=wv_sb[:, hc, :],
                    start=(hc == 0), stop=False)
            nc.tensor.matmul(
                acc, lhsT=ones_col, rhs=bv_row, start=False, stop=True)
            nc.vector.tensor_copy(
                out=v_aug[:, tt, :, 0:HD],
                in_=acc.rearrange("p (h d) -> p h d", h=NHL))

        def emit_qk_group(which, ct, nq):
            w_sb, b_sb, dst = ((wq_sb, bq_sb, qT), (wk_sb, bk_sb, kT))[which]
            acc = pps.tile([P, MM_N], F32, tag="acc")
            for hc in range(HC):
                nc.tensor.matmul(
                    acc,
                    lhsT=w_sb[:, hc, ct * P:(ct + 1) * P],
                    rhs=xT[:, hc, nq * MM_N:(nq + 1) * MM_N],
                    start=(hc == 0), stop=(hc == HC - 1))
            nc.vector.tensor_scalar_add(
                out=dst[:, ct, nq * MM_N:(nq + 1) * MM_N],
                in0=acc, scalar1=b_sb[:, ct:ct + 1])

        osb = ctx.enter_context(tc.tile_pool(name="o_sb", bufs=3))

        def emit_o_slice(tt):
            accs = [pps.tile([P, MM_N], F32, tag="acc", name=f"oacc{j}")
                    for j in range(2)]
            for cc in range(CT):
                for no in range(H // MM_N):
                    nc.tensor.matmul(
                        accs[no],
                        lhsT=ctxT[:, cc, tt * P:(tt + 1) * P],
                        rhs=wo_sb[:, cc, no * MM_N:(no + 1) * MM_N],
                        start=(cc == 0), stop=(cc == CT - 1))
            for no in range(H // MM_N):
                ot = osb.tile([P, MM_N], F32)
                nc.vector.tensor_copy(out=ot, in_=accs[no])
                nc.sync.dma_start(
                    out=out[tt * P:(tt + 1) * P, no * MM_N:(no + 1) * MM_N],
                    in_=ot)

        with tc.tile_pool(name="expp", bufs=3) as exp_pool, \
             tc.tile_pool(name="normp", bufs=2) as norm_pool:
            # qk projections for c-tile 0 upfront (first scores need them)
            for which in (1, 0):
                for nq in range(S // MM_N):
                    emit_qk_group(which, 0, nq)

            for ct in range(CT):
                # fill work to drip through this c-tile's four windows: the
                # qk projections of the NEXT c-tile (8 groups over 64 slots)
                fill = [(which, ct + 1, nq)
                        for which in (1, 0)
                        for nq in range(S // MM_N)] if ct + 1 < CT else []
                slot = 0
                for h in (2 * ct, 2 * ct + 1):
                    po = (h % 2) * HD
                    for qh in range(NQH):
                        w_idx = h * NQH + qh
                        q0 = qh * QW
                        ctx_ps = ctx_pool.tile([HD + 1, QW], F32)
                        prev_ex = None
                        for kc in range(KC):
                            # PE fill-in: v projection rides window 0; next
                            # c-tile's qk projections drip one group per 8 kc
                            if ct == 0 and h == 0 and qh == 0:
                                emit_v_slice(kc)
                            if fill and slot % 8 == 4:
                                emit_qk_group(*fill.pop(0))
                            slot += 1
                            sc = sc_pool.tile([P, QW], F32)
                            for nq in range(QW // MM_N):
                                nc.tensor.matmul(
                                    sc[:, nq * MM_N:(nq + 1) * MM_N],
                                    lhsT=kT[po:po + HD, ct, kc * P:(kc + 1) * P],
                                    rhs=qT[po:po + HD, ct,
                                           q0 + nq * MM_N:q0 + (nq + 1) * MM_N],
                                    start=True, stop=True)
                            if prev_ex is not None:
                                pex, pkc = prev_ex
                                for nq in range(QW // MM_N):
                                    nc.tensor.matmul(
                                        ctx_ps[:, nq * MM_N:(nq + 1) * MM_N],
                                        lhsT=v_aug[:, pkc, h, :],
                                        rhs=pex[:, nq * MM_N:(nq + 1) * MM_N],
                                        start=(pkc == 0), stop=False)
                            ex = exp_pool.tile([P, QW], BF16)
                            nc.scalar.activation(
                                out=ex, in_=sc,
                                func=mybir.ActivationFunctionType.Exp,
                                scale=SCALE)
                            prev_ex = (ex, kc)
                        pex, pkc = prev_ex
                        for nq in range(QW // MM_N):
                            nc.tensor.matmul(
                                ctx_ps[:, nq * MM_N:(nq + 1) * MM_N],
                                lhsT=v_aug[:, pkc, h, :],
                                rhs=pex[:, nq * MM_N:(nq + 1) * MM_N],
                                start=False, stop=True)
                        # park raw ctx (bf16); reciprocal of the denominator
                        # row via ACT exp(-ln(x)) straight from psum, then
                        # partition-broadcast through DRAM and multiply in
                        # place -- all off the PE critical path
                        rs_row = norm_pool.tile([1, QW], F32, tag="rs")
                        nc.vector.tensor_copy(
                            out=rs_row, in_=ctx_ps[HD:HD + 1, :])
                        nc.vector.tensor_copy(
                            out=ctxT[po:po + HD, ct, q0:q0 + QW],
                            in_=ctx_ps[0:HD, :])
                        rcp_f = norm_pool.tile([1, QW], F32, tag="rcpf")
                        nc.vector.reciprocal(out=rcp_f, in_=rs_row)
                        rcp_row = norm_pool.tile([1, QW], BF16, tag="rcp")
                        nc.vector.tensor_copy(out=rcp_row, in_=rcp_f)
                        nc.sync.dma_start(
                            out=rcp_dram[w_idx:w_idx + 1, :], in_=rcp_row)
                        row = rcp_dram[w_idx:w_idx + 1, :]
                        bcast = norm_pool.tile([P, QW], BF16, tag="bcast")
                        nc.sync.dma_start(
                            out=bcast[po:po + HD, :],
                            in_=bass.AP(tensor=row.tensor, offset=row.offset,
                                        ap=[[0, HD], [1, QW]]))
                        sl = ctxT[po:po + HD, ct, q0:q0 + QW]
                        nc.vector.tensor_mul(
                            out=sl, in0=sl, in1=bcast[po:po + HD, :])


        # ---- phase 3: output projection (partial; host sums group halves) ----
        for tt in range(TT):
            emit_o_slice(tt)


def build_program():
    global _NC_CACHE
    if _NC_CACHE is not None:
        return _NC_CACHE
    nc = bacc.Bacc("TRN2", debug=False, num_devices=8)
    x = nc.dram_tensor("x", [S, H], F32, kind="ExternalInput").ap()
    wq = nc.dram_tensor("wq", [H, G], F32, kind="ExternalInput").ap()
    wk = nc.dram_tensor("wk", [H, G], F32, kind="ExternalInput").ap()
    wv = nc.dram_tensor("wv", [H, G], F32, kind="ExternalInput").ap()
    wo = nc.dram_tensor("wo", [G, H], F32, kind="ExternalInput").ap()
    bq = nc.dram_tensor("bq", [G], F32, kind="ExternalInput").ap()
    bk = nc.dram_tensor("bk", [G], F32, kind="ExternalInput").ap()
    bv = nc.dram_tensor("bv", [G], F32, kind="ExternalInput").ap()
    out = nc.dram_tensor("out", [S, H], F32, kind="ExternalOutput").ap()
    rcp_dram = nc.dram_tensor("rcp_scratch", [NW, QW], BF16).ap()
    with tile.TileContext(nc) as tc:
        _emit(nc, tc, (x, wq, wk, wv, wo, bq, bk, bv, out, rcp_dram))
    nc.compile()
    _NC_CACHE = nc
    return nc


def make_in_maps(x, wq, bq, wk, bk, wv, bv, wo, bo):
    x = np.asarray(x, dtype=np.float32)
    in_maps = []
    for c in range(8):
        b, g = divmod(c, 2)
        sl = slice(g * G, (g + 1) * G)
        in_maps.append({
            "x": np.ascontiguousarray(x[b]),
            "wq": np.ascontiguousarray(np.asarray(wq, np.float32)[:, sl]),
            "wk": np.ascontiguousarray(np.asarray(wk, np.float32)[:, sl]),
            "wv": np.ascontiguousarray(np.asarray(wv, np.float32)[:, sl]),
            "wo": np.ascontiguousarray(np.asarray(wo, np.float32)[sl, :]),
            "bq": np.ascontiguousarray(np.asarray(bq, np.float32)[sl]),
            "bk": np.ascontiguousarray(np.asarray(bk, np.float32)[sl]),
            "bv": np.ascontiguousarray(np.asarray(bv, np.float32)[sl]),
        })
    return in_maps


def gather_out(results, bo):
    bo = np.asarray(bo, dtype=np.float32)
    out = np.empty((B, S, H), dtype=np.float32)
    for b in range(B):
        out[b] = results[2 * b]["out"] + results[2 * b + 1]["out"] + bo
    return out


def kernel(x, wq, bq, wk, bk, wv, bv, wo, bo, trace=False):
    nc = build_program()
    in_maps = make_in_maps(x, wq, bq, wk, bk, wv, bv, wo, bo)
    r = run_bass_kernel_spmd(nc, in_maps, list(range(8)), trace=trace)
    out = gather_out(r.results, bo)
    if trace:
        kernel.last_exec_time_ns = r.exec_time_ns
        kernel.last_results = r
    return out
